# revision 1
# baseline (speedup 1.0000x reference)
"""BlocksGNN message-passing kernel for Trainium2 (Bass/Tile), 8-core data-parallel.

Math restructuring (done host-side in numpy, inside kernel()):
  - edge MLP layer 1 on concat(src,tgt) splits into per-node halves:
        A = node @ ew1[:D], B = node @ ew1[D:]        (14 node-cols instead of 49 edge-cols)
  - LayerNorm mean-subtraction folds into column-centered W2/b2
    (LN(Wx+b) centering: h - mean_f(h) = (W - rowwise-mean W)x + (b - mean b))
  - LN gain g folds into W2/b2; variance recovered with per-feature 1/g^2 weights
  - pass-1 edge layer-3 commutes with the j-sum and fuses into the node MLP:
        agg @ nw1b = (sum_j e_r) @ (ew3 @ nw1b) = sagg @ W_agg
  - pass-2 edge layer-3 + head fuse into a single vector v = ew3 @ few

On-device layout is feature-major: features on SBUF partitions (4 tiles of 128),
edge/node columns along the free dim.  All matmuls then take the weights as lhsT
in their natural [in,out] layout and need no activation transposes; the only
transpose is the initial states load (PE transpose via identity).
"""

import sys

try:
    import concourse.bass as bass  # noqa: F401
except ImportError:
    sys.path.insert(0, "/opt/trn_rl_repo")

import contextlib

import numpy as np

import concourse.bacc as bacc
import concourse.bass as bass
import concourse.mybir as mybir
import concourse.tile as tile
from concourse import masks
from concourse.bass_utils import run_bass_kernel_spmd

F32 = mybir.dt.float32
AF = mybir.ActivationFunctionType
ALU = mybir.AluOpType

LN_EPS = 1e-5

# problem geometry (hardcoded per harness contract)
N_CORES = 8
B_FULL = 4096
NNODE = 7
D = 512
H = 512
KT = 4          # 512 // 128 feature tiles
NEDGE = NNODE * NNODE

# vecs_kt row indices
V_EB1, V_EBT, V_WSSE, V_NBT, V_WSSN, V_V, V_FNW = range(7)
V_EB2, V_NB1, V_NB2, V_NB3 = 7, 8, 9, 10
NV = 11


class Cfg:
    def __init__(self, b_core=512, sb=64, c=8, mm_f32r=True,
                 zeros=frozenset(), fnb=0.0, c_e=0.0):
        self.b_core = b_core      # graphs per core
        self.sb = sb              # graphs per super-block
        self.c = c                # graphs per edge chunk
        self.mm_f32r = mm_f32r    # use float32r matmuls (4x faster, slightly lossy)
        self.zeros = zeros        # which bias vectors are all-zero (build-time fold)
        self.fnb = fnb            # node-head scalar bias
        self.c_e = c_e            # edge-head scalar bias
        assert b_core % sb == 0 and sb % c == 0
        self.nsb = b_core // sb
        self.nch = sb // c
        self.ncols = sb * NNODE       # node cols per super-block
        self.ecols = c * NEDGE        # edge cols per chunk

    def key(self):
        return (self.b_core, self.sb, self.c, self.mm_f32r,
                tuple(sorted(self.zeros)), self.fnb, self.c_e,
                getattr(self, "rep", 1))


def build_program(cfg: Cfg):
    """Build the per-core Bass program."""
    nc = bacc.Bacc("TRN2", target_bir_lowering=False, debug=False)

    b, sb, c = cfg.b_core, cfg.sb, cfg.c
    ncols, ecols = cfg.ncols, cfg.ecols
    F32R = mybir.dt.float32r if cfg.mm_f32r else F32

    # ---- DRAM I/O ----
    states_d = nc.dram_tensor("states", [b * NNODE, D], F32, kind="ExternalInput").ap()
    wnames = ["w_e1a", "w_e1b", "w_e2", "w_agg", "w_n1a", "w_n2", "w_n3"]
    wd = {n: nc.dram_tensor(n, [D, H], F32R, kind="ExternalInput").ap()
          for n in wnames}
    vecs_d = nc.dram_tensor("vecs_kt", [NV, H], F32, kind="ExternalInput").ap()
    vecsr_d = nc.dram_tensor("vecs_r", [NV, H], F32R, kind="ExternalInput").ap()
    out_d = nc.dram_tensor("out", [b, NNODE + NEDGE], F32, kind="ExternalOutput").ap()

    dbg_taps = []

    def tap(name, ap):
        if getattr(cfg, "debug", False):
            dbg_taps.append((name, ap))

    with tile.TileContext(nc) as tc, contextlib.ExitStack() as ctx:
        singles = ctx.enter_context(tc.tile_pool(name="singles", bufs=1))
        p_node = ctx.enter_context(tc.tile_pool(name="p_node", bufs=2))
        p_ab1 = ctx.enter_context(tc.tile_pool(name="p_ab1", bufs=2))
        p_ab2 = ctx.enter_context(tc.tile_pool(name="p_ab2", bufs=1))
        p_sagg = ctx.enter_context(tc.tile_pool(name="p_sagg", bufs=1))
        p_node2 = ctx.enter_context(tc.tile_pool(name="p_node2", bufs=1))
        p_stage = ctx.enter_context(tc.tile_pool(name="p_stage", bufs=2))
        p_epre = ctx.enter_context(tc.tile_pool(name="p_epre", bufs=2))
        fast_build = all(i in cfg.zeros for i in (V_EB2, V_EBT, V_NB2, V_NBT))
        p_epre0 = ctx.enter_context(
            tc.tile_pool(name="p_epre0", bufs=2 if fast_build else 1))
        p_h = ctx.enter_context(tc.tile_pool(name="p_h", bufs=2))
        p_sq = ctx.enter_context(tc.tile_pool(name="p_sq", bufs=1))
        p_tm = ctx.enter_context(tc.tile_pool(name="p_tm", bufs=1))
        p_bc = ctx.enter_context(tc.tile_pool(name="p_bc", bufs=2))
        p_small = ctx.enter_context(tc.tile_pool(name="p_small", bufs=2))
        p_out = ctx.enter_context(tc.tile_pool(name="p_out", bufs=1))
        # PSUM: per-m-tile 1-bank tiles; 5 slots so evacuation overlaps matmuls
        ps_mm = ctx.enter_context(tc.tile_pool(name="ps_mm", bufs=2, space="PSUM"))
        ps_var = ctx.enter_context(tc.tile_pool(name="ps_var", bufs=2, space="PSUM"))
        ps_bc = ctx.enter_context(tc.tile_pool(name="ps_bc", bufs=2, space="PSUM"))

        # ---- constants ----
        ident = singles.tile([128, 128], F32)
        masks.make_identity(nc, ident[:, :])

        ws = {}
        for n in wnames:
            wt = singles.tile([128, KT, H], F32R, name=f"sb_{n}")
            nc.sync.dma_start(out=wt, in_=wd[n].rearrange("(k p) m -> p k m", p=128))
            ws[n] = wt

        vecs = singles.tile([128, NV, KT], F32)
        nc.sync.dma_start(out=vecs, in_=vecs_d.rearrange("v (k p) -> p v k", p=128))
        vecs_r = singles.tile([128, NV, KT], F32R)
        nc.sync.dma_start(out=vecs_r,
                          in_=vecsr_d.rearrange("v (k p) -> p v k", p=128))
        ones_f = singles.tile([1, 128], F32)
        nc.vector.memset(ones_f, 1.0)
        ones_col = singles.tile([1, 128], F32R)
        nc.vector.tensor_copy(ones_col, ones_f)
        eps_t = singles.tile([1, 1], F32)
        nc.vector.memset(eps_t, LN_EPS)
        eps_r = singles.tile([1, 1], F32R)
        nc.vector.tensor_copy(eps_r, eps_t)
        ones_row_f = singles.tile([1, 448], F32)
        nc.vector.memset(ones_row_f, 1.0)
        ones_row = singles.tile([1, 448], F32R)
        nc.vector.tensor_copy(ones_row, ones_row_f)

        def load_w(name, k, m):
            return ws[name][:, k, m * 128:(m + 1) * 128]

        def vslice(v, k):
            return vecs_r[:, v, k:k + 1]

        def mm_group(rhs_tiles_fn, wname, n, extra=None):
            """Per-m-pair accumulation: returns 2 two-bank PSUM tiles
            [128, 2, 512] (pair p holds out-tiles 2p, 2p+1)."""
            psums = []
            for p in range(2):
                ps = ps_mm.tile([128, 2, 512], F32, tag="mm", name=f"ps_mm{p}")
                for mi in range(2):
                    m = 2 * p + mi
                    for k in range(KT):
                        nc.tensor.matmul(
                            ps[:, mi, 0:n], load_w(wname, k, m), rhs_tiles_fn(k),
                            start=(k == 0), stop=(k == KT - 1 and extra is None))
                    if extra is not None:
                        wname2, rhs2_fn = extra
                        for k in range(KT):
                            nc.tensor.matmul(
                                ps[:, mi, 0:n], load_w(wname2, k, m), rhs2_fn(k),
                                start=False, stop=(k == KT - 1))
                psums.append(ps)
            return psums

        def ln_tail(psums, n, wss_idx, bias_idx, relu_bias_idx, h_sb, sq, bc_sb):
            """pre-LN h in per-m PSUMs -> h_sb (relu'd on fast path), rstd bc.

            Fast path: h_sb = relu(h) via relu(h*rstd) = relu(h)*rstd."""
            fast = (bias_idx in cfg.zeros_idx
                    and relu_bias_idx in cfg.zeros_idx)
            for p in range(2):
                hv = h_sb[:, 2 * p:2 * p + 2, 0:n]
                pv = psums[p][:, :, 0:n]
                if fast:
                    # evac pair0 on ACT, pair1 on DVE; squares on ACT
                    if p == 0:
                        nc.scalar.activation(hv, pv, AF.Relu)
                    else:
                        nc.vector.tensor_scalar_max(hv, pv, 0.0)
                    nc.scalar.square(sq[:, 2 * p:2 * p + 2, 0:n], pv)
                else:
                    for mi in range(2):
                        m = 2 * p + mi
                        nc.vector.tensor_scalar_add(
                            h_sb[:, m, 0:n], psums[p][:, mi, 0:n],
                            vecs[:, bias_idx, m:m + 1])
                    nc.scalar.square(sq[:, 2 * p:2 * p + 2, 0:n],
                                     h_sb[:, 2 * p:2 * p + 2, 0:n])
            psum_var = ps_var.tile([1, 512], F32, tag="var")
            for k in range(KT):
                nc.tensor.matmul(
                    psum_var[0:1, 0:n], vslice(wss_idx, k), sq[:, k, 0:n],
                    start=(k == 0), stop=False)
            nc.tensor.matmul(psum_var[0:1, 0:n], eps_r[0:1, 0:1],
                             ones_row[0:1, 0:n], start=False, stop=True)
            inv_v = p_small.tile([1, 448], F32, tag="s_sb", bufs=1)
            nc.vector.reciprocal_approx_fast(inv_v[0:1, 0:n],
                                             psum_var[0:1, 0:n])
            rstd_r = p_small.tile([1, 448], F32R, tag="rstd_r")
            nc.scalar.activation(rstd_r[0:1, 0:n], inv_v[0:1, 0:n], AF.Sqrt)
            psum_b = ps_bc.tile([128, 512], F32, tag="bc")
            nc.tensor.matmul(psum_b[:, 0:n], ones_col[0:1, :],
                             rstd_r[0:1, 0:n], start=True, stop=True)
            nc.scalar.copy(bc_sb[:, 0:n], psum_b[:, 0:n])
            return fast

        def ln_apply(fast, h_sb, bc_sb, n, relu_bias_idx, out_tile):
            """out = relu((h*rstd) + relu_bias); fast path: relu(h)*rstd."""
            bcb = bc_sb.unsqueeze(1).broadcast_to([128, KT, n])
            if fast:
                # both operands in SBUF -> GPSIMD (keeps DVE free)
                nc.gpsimd.tensor_mul(out_tile[:, :, 0:n], h_sb[:, :, 0:n], bcb)
            else:
                tmf = p_tm.tile([128, KT, 448], F32, tag="tmf")
                nc.vector.tensor_mul(tmf[:, :, 0:n], h_sb[:, :, 0:n], bcb)
                for k in range(KT):
                    nc.vector.tensor_scalar(
                        out=out_tile[:, k, 0:n], in0=tmf[:, k, 0:n],
                        scalar1=vecs[:, relu_bias_idx, k:k + 1], scalar2=0.0,
                        op0=ALU.add, op1=ALU.max)

        def edge_front(a_t, b_t, ch, dst_pre):
            """dst_pre = relu(A[i] + B[j])   (eb1 folded into A)."""
            c0 = ch * c * NNODE
            epre0 = p_epre0.tile([128, KT, ecols], F32, tag="epre0")
            for k in range(KT):
                a_ap = (a_t[:, k, c0:c0 + c * NNODE]
                        .rearrange("p (g i) -> p g i", i=NNODE)
                        .unsqueeze(3).broadcast_to([128, c, NNODE, NNODE]))
                b_ap = (b_t[:, k, c0:c0 + c * NNODE]
                        .rearrange("p (g j) -> p g j", j=NNODE)
                        .unsqueeze(2).broadcast_to([128, c, NNODE, NNODE]))
                o_ap = epre0[:, k, :].rearrange("p (g i j) -> p g i j",
                                                i=NNODE, j=NNODE)
                eng = nc.vector if k < 2 else nc.gpsimd
                eng.tensor_add(o_ap, a_ap, b_ap)
            nc.gpsimd.tensor_scalar_max(dst_pre, epre0, 0.0)

        def compute_ab(src_t, pool):
            """A/B = src @ ew1 halves, with eb1 folded into A."""
            a_t = pool.tile([128, KT, ncols], F32, tag="a_t")
            b_t = pool.tile([128, KT, ncols], F32, tag="b_t")
            for wn, dst, bias_idx in [("w_e1a", a_t, V_EB1), ("w_e1b", b_t, None)]:
                psums = mm_group(lambda k: src_t[:, k, 0:ncols], wn, ncols)
                for p in range(2):
                    dv = dst[:, 2 * p:2 * p + 2, 0:ncols]
                    pv = psums[p][:, :, 0:ncols]
                    if bias_idx is not None and bias_idx not in cfg.zeros_idx:
                        for mi in range(2):
                            m = 2 * p + mi
                            nc.vector.tensor_scalar_add(
                                dst[:, m, 0:ncols], psums[p][:, mi, 0:ncols],
                                vecs[:, bias_idx, m:m + 1])
                    elif p == 0:
                        nc.scalar.copy(dv, pv)
                    else:
                        nc.vector.tensor_copy(dv, pv)
            return a_t, b_t

        def make_front(a_t, b_t, ch):
            epre = p_epre.tile([128, KT, ecols], F32R, tag="epre")
            edge_front(a_t, b_t, ch, epre)
            return epre

        def edge_chunk(epre, front_next, ch, sagg, out_head):
            """One pass-1 (sagg) or pass-2 (out_head) edge chunk.

            front_next() emits the next chunk's assembly right after this
            chunk's matmuls so the DVE work for c+1 schedules ahead of this
            chunk's LN tail (keeps PE fed)."""
            psums = mm_group(lambda k: epre[:, k, 0:ecols], "w_e2", ecols)
            nxt = front_next() if front_next else None
            h_sb = p_h.tile([128, KT, ecols], F32, tag="h")
            sq = p_sq.tile([128, KT, ecols], F32R, tag="sq")
            bc_sb = p_bc.tile([128, ecols], F32, tag="bcs")
            fast = ln_tail(psums, ecols, V_WSSE, V_EB2, V_EBT, h_sb, sq, bc_sb)
            if sagg is not None:  # pass 1: j-sum into sagg columns
                tm = p_tm.tile([128, KT, ecols], F32, tag="tm1")
                ln_apply(fast, h_sb, bc_sb, ecols, V_EBT, tm)
                with nc.allow_low_precision(reason="f32r round of f32 sum"):
                    nc.vector.tensor_reduce(
                        sagg[:, :, ch * c * NNODE:(ch + 1) * c * NNODE],
                        tm.rearrange("p k (n j) -> p k n j", j=NNODE),
                        axis=mybir.AxisListType.X, op=ALU.add)
            else:  # pass 2: edge head
                tm = p_tm.tile([128, KT, ecols], F32R, tag="tm2")
                ln_apply(fast, h_sb, bc_sb, ecols, V_EBT, tm)
                s, g0 = out_head
                psum_eo = ps_var.tile([1, 512], F32, tag="var")
                for k in range(KT):
                    nc.tensor.matmul(psum_eo[0:1, 0:ecols], vslice(V_V, k),
                                     tm[:, k, 0:ecols],
                                     start=(k == 0), stop=(k == KT - 1))
                eo_sb = p_out.tile([1, 448], F32, tag="head_sb")
                nc.scalar.activation(eo_sb[0:1, 0:ecols], psum_eo[0:1, 0:ecols],
                                     AF.Copy, bias=cfg.c_e)
                nc.sync.dma_start(
                    out=out_d[g0:g0 + c, NNODE:NNODE + NEDGE].unsqueeze(0),
                    in_=eo_sb[0:1, 0:ecols].rearrange("o (g e) -> o g e",
                                                      e=NEDGE))
            return nxt

        # =========================== main loop ===========================
        for _rep in range(getattr(cfg, "rep", 1)):
          for s in range(cfg.nsb):
              # load + transpose states
              node_t = p_node.tile([128, KT, ncols], F32R, tag="node_t")
              r0 = s * ncols
              tcols = 112
              for t in range(ncols // tcols):
                  stg = p_stage.tile([tcols, D], F32, tag="stage")
                  nc.sync.dma_start(
                      out=stg, in_=states_d[r0 + t * tcols: r0 + (t + 1) * tcols, :])
                  psum_t = ps_mm.tile([128, 2, 512], F32, tag="mm", name="ps_tp")
                  tpv = psum_t.rearrange("p a b -> p (a b)")[:, 0:KT * tcols] \
                      .rearrange("p (m q) -> p m q", q=tcols)
                  for m in range(KT):
                      nc.tensor.transpose(
                          tpv[:, m, :], stg[:, m * 128:(m + 1) * 128],
                          ident[0:tcols, 0:tcols])
                  nc.vector.tensor_copy(
                      node_t[:, :, t * tcols:(t + 1) * tcols], tpv)

              a1_t, b1_t = compute_ab(node_t, p_ab1)

              sagg = p_sagg.tile([128, KT, ncols], F32R, tag="sagg")
              epre = make_front(a1_t, b1_t, 0)
              for ch in range(cfg.nch):
                  nf = ((lambda cc=ch: make_front(a1_t, b1_t, cc + 1))
                        if ch + 1 < cfg.nch else None)
                  epre = edge_chunk(epre, nf, ch, sagg, None)

              # node MLP layer 1 (node_t @ nw1a + sagg @ w_agg, fused accumulation)
              psums = mm_group(lambda k: node_t[:, k, 0:ncols], "w_n1a", ncols,
                               extra=("w_agg", lambda k: sagg[:, k, 0:ncols]))
              nh1 = p_epre.tile([128, KT, ncols], F32R, tag="epre")
              for p in range(2):
                  nv = nh1[:, 2 * p:2 * p + 2, 0:ncols]
                  pv = psums[p][:, :, 0:ncols]
                  if V_NB1 in cfg.zeros_idx:
                      if p == 0:
                          nc.scalar.activation(nv, pv, AF.Relu)
                      else:
                          nc.vector.tensor_scalar_max(nv, pv, 0.0)
                  else:
                      for mi in range(2):
                          m = 2 * p + mi
                          nc.scalar.activation(
                              nh1[:, m, 0:ncols], psums[p][:, mi, 0:ncols],
                              AF.Relu, bias=vecs[:, V_NB1, m:m + 1])

              psums = mm_group(lambda k: nh1[:, k, 0:ncols], "w_n2", ncols)
              nh_sb = p_h.tile([128, KT, ncols], F32, tag="h")
              nsq = p_sq.tile([128, KT, ncols], F32R, tag="sq")
              nbc = p_bc.tile([128, ncols], F32, tag="bcs")
              nfast = ln_tail(psums, ncols, V_WSSN, V_NB2, V_NBT, nh_sb, nsq, nbc)
              ntm = p_tm.tile([128, KT, ncols], F32R, tag="tm2")
              ln_apply(nfast, nh_sb, nbc, ncols, V_NBT, ntm)

              psums = mm_group(lambda k: ntm[:, k, 0:ncols], "w_n3", ncols)
              node2_t = p_node2.tile([128, KT, ncols], F32R, tag="node2")
              for p in range(2):
                  nv = node2_t[:, 2 * p:2 * p + 2, 0:ncols]
                  pv = psums[p][:, :, 0:ncols]
                  if V_NB3 in cfg.zeros_idx:
                      if p == 0:
                          nc.scalar.copy(nv, pv)
                      else:
                          nc.vector.tensor_copy(nv, pv)
                  else:
                      for mi in range(2):
                          m = 2 * p + mi
                          nc.vector.tensor_scalar_add(
                              node2_t[:, m, 0:ncols], psums[p][:, mi, 0:ncols],
                              vecs[:, V_NB3, m:m + 1])

              # node head -> out[:, 0:7]
              psum_no = ps_var.tile([1, 512], F32, tag="var")
              for k in range(KT):
                  nc.tensor.matmul(psum_no[0:1, 0:ncols], vslice(V_FNW, k),
                                   node2_t[:, k, 0:ncols],
                                   start=(k == 0), stop=(k == KT - 1))
              no_sb = p_out.tile([1, 448], F32, tag="head_sb")
              nc.scalar.activation(no_sb[0:1, 0:ncols], psum_no[0:1, 0:ncols],
                                   AF.Copy, bias=cfg.fnb)
              nc.sync.dma_start(
                  out=out_d[s * sb:(s + 1) * sb, 0:NNODE].unsqueeze(0),
                  in_=no_sb[0:1, 0:ncols].rearrange("o (g i) -> o g i", i=NNODE))

              # pass 2
              a2_t, b2_t = compute_ab(node2_t, p_ab2)
              epre = make_front(a2_t, b2_t, 0)
              for ch in range(cfg.nch):
                  nf = ((lambda cc=ch: make_front(a2_t, b2_t, cc + 1))
                        if ch + 1 < cfg.nch else None)
                  epre = edge_chunk(epre, nf, ch, None, (s, s * sb + ch * c))

        for name, ap in dbg_taps:
            shp = list(ap.shape)
            dd = nc.dram_tensor(f"dbg_{name}", shp, F32, kind="ExternalOutput").ap()
            nc.sync.dma_start(out=dd, in_=ap.bitcast(mybir.dt.float32))

    nc.compile()
    return nc

def host_fold(inputs):
    """Numpy pre-folding of weights. Returns (tensors, zeros-set, fnb, c_e)."""
    f = lambda k: np.asarray(inputs[k], np.float64)
    ew1, eb1, ew2, eb2 = f("ew1"), f("eb1"), f("ew2"), f("eb2")
    eg, ebt, ew3, eb3 = f("eg"), f("ebt"), f("ew3"), f("eb3")
    nw1, nb1, nw2, nb2 = f("nw1"), f("nb1"), f("nw2"), f("nb2")
    ng, nbt, nw3, nb3 = f("ng"), f("nbt"), f("nw3"), f("nb3")
    fnw, fnb, few, feb = f("fnw"), f("fnb"), f("few"), f("feb")

    ew2c = ew2 - ew2.mean(axis=1, keepdims=True)
    eb2cg = (eb2 - eb2.mean()) * eg
    ew2cg = ew2c * eg[None, :]
    wss_e = 1.0 / np.maximum(eg * eg, 1e-12) / H

    nw1a, nw1b = nw1[:D], nw1[D:]
    w_agg = ew3 @ nw1b
    nb1p = nb1 + 7.0 * (eb3 @ nw1b)
    nw2c = nw2 - nw2.mean(axis=1, keepdims=True)
    nb2cg = (nb2 - nb2.mean()) * ng
    nw2cg = nw2c * ng[None, :]
    wss_n = 1.0 / np.maximum(ng * ng, 1e-12) / H

    v = (ew3 @ few)[:, 0]
    c_e = float(eb3 @ few[:, 0] + feb[0])

    g = lambda x: np.ascontiguousarray(x, np.float32)
    vec_rows = [eb1, ebt, wss_e, nbt, wss_n, v, fnw[:, 0], eb2cg, nb1p, nb2cg, nb3]
    vecs_kt = g(np.stack(vec_rows))

    zeros = frozenset(
        i for i in (V_EB1, V_EBT, V_NBT, V_EB2, V_NB1, V_NB2, V_NB3)
        if not np.any(vec_rows[i]))

    tensors = {
        "w_e1a": g(ew1[:D]), "w_e1b": g(ew1[D:]), "w_e2": g(ew2cg),
        "w_agg": g(w_agg), "w_n1a": g(nw1a), "w_n2": g(nw2cg), "w_n3": g(nw3),
        "vecs_kt": vecs_kt, "vecs_r": vecs_kt,
    }
    return tensors, zeros, float(fnb[0]), c_e


_CACHE = {}


def get_program(cfg: Cfg):
    # build-time specialization needs zeros visible inside build_program
    cfg.zeros_idx = cfg.zeros
    key = cfg.key()
    if key not in _CACHE:
        _CACHE[key] = build_program(cfg)
    return _CACHE[key]


def kernel(**inputs) -> np.ndarray:
    states = np.asarray(inputs["states"], np.float32)
    B, n, d = states.shape
    assert (B, n, d) == (B_FULL, NNODE, D)

    folded, zeros, fnb, c_e = host_fold(inputs)
    cfg = Cfg(b_core=B // N_CORES, zeros=zeros, fnb=fnb, c_e=c_e)
    nc = get_program(cfg)

    in_maps = []
    for ci in range(N_CORES):
        m = dict(folded)
        m["states"] = np.ascontiguousarray(
            states[ci * cfg.b_core:(ci + 1) * cfg.b_core].reshape(-1, D))
        in_maps.append(m)

    res = run_bass_kernel_spmd(nc, in_maps, list(range(N_CORES)))
    return np.concatenate([r["out"] for r in res.results], axis=0)



# revision 19
# speedup vs baseline: 1.4192x; 1.4192x over previous
"""BlocksGNN message-passing kernel for Trainium2 (Bass/Tile), 8-core data-parallel.

Math restructuring (done host-side in numpy, inside kernel()):
  - edge MLP layer 1 on concat(src,tgt) splits into per-node halves:
        A = node @ ew1[:D], B = node @ ew1[D:]        (14 node-cols instead of 49 edge-cols)
  - LayerNorm mean-subtraction folds into column-centered W2/b2
    (LN(Wx+b) centering: h - mean_f(h) = (W - rowwise-mean W)x + (b - mean b))
  - LN gain g folds into W2/b2; variance recovered with per-feature 1/g^2 weights
  - pass-1 edge layer-3 commutes with the j-sum and fuses into the node MLP:
        agg @ nw1b = (sum_j e_r) @ (ew3 @ nw1b) = sagg @ W_agg
  - pass-2 edge layer-3 + head fuse into a single vector v = ew3 @ few

On-device layout is feature-major: features on SBUF partitions (4 tiles of 128),
edge/node columns along the free dim.

v2 fast path (requires all-zero biases + unit gains, which the harness fills
guarantee; falls back to the v1 generic path otherwise):
  - weights and activations in bf16 (PE matmul rate is the same as f32r at
    >=256-col streams, but DVE tensor ops get the 2x 2-byte mode and SBUF
    footprint halves)
  - LN variance via fp8 squares + DoubleRow matmuls (4x fewer PE cycles):
    sq = (h*s)^2 in fp8e4m3 with per-layer scale s centering values in the
    fp8 normal range; DR lhsT = 1.0 exactly; the resulting constant
    C = s*sqrt(H) folds into the single downstream consumer of each LN
    output (w_agg / nw3 / v), and the eps matmul row carries eps*H*s^2
  - pass-2 skips the LN-apply entirely: eo = (v . relu(h)) * rstd + c_e
  - rstd broadcast via gpsimd.partition_broadcast (no PE ones-matmul, no
    ACT PSUM-evac copy)
  - states transposes in f32r (1.5 PE cycles/row instead of 2.0)
"""

import sys

try:
    import concourse.bass as bass  # noqa: F401
except ImportError:
    sys.path.insert(0, "/opt/trn_rl_repo")

import contextlib

import ml_dtypes
import numpy as np

import concourse.bacc as bacc
import concourse.bass as bass
import concourse.mybir as mybir
import concourse.tile as tile
from concourse import masks
from concourse.bass_utils import run_bass_kernel_spmd

F32 = mybir.dt.float32
F32R = mybir.dt.float32r
BF16 = mybir.dt.bfloat16
FP8 = mybir.dt.float8e4
AF = mybir.ActivationFunctionType
ALU = mybir.AluOpType
DR = mybir.MatmulPerfMode.DoubleRow
NPBF16 = ml_dtypes.bfloat16

LN_EPS = 1e-5

# problem geometry (hardcoded per harness contract)
N_CORES = 8
B_FULL = 4096
NNODE = 7
D = 512
H = 512
KT = 4          # 512 // 128 feature tiles
NEDGE = NNODE * NNODE

# fp8-square scales per LN site (s.t. (h*s)^2 sits in fp8 normal range;
# empirical max|h| is ~2.4 for edge-pass-1/node and ~0.5 for edge-pass-2)
S_E1 = 4.0
S_N = 4.0
S_E2 = 8.0

# v1 vecs_kt row indices
V_EB1, V_EBT, V_WSSE, V_NBT, V_WSSN, V_V, V_FNW = range(7)
V_EB2, V_NB1, V_NB2, V_NB3 = 7, 8, 9, 10
NV = 11


class Cfg:
    def __init__(self, b_core=512, sb=64, c=8, mm_f32r=True,
                 zeros=frozenset(), fnb=0.0, c_e=0.0, v2=False):
        self.b_core = b_core      # graphs per core
        self.sb = sb              # graphs per super-block
        self.c = c                # graphs per edge chunk
        self.mm_f32r = mm_f32r    # v1: use float32r matmuls
        self.zeros = zeros        # which bias vectors are all-zero
        self.fnb = fnb            # node-head scalar bias
        self.c_e = c_e            # edge-head scalar bias
        self.v2 = v2
        assert b_core % sb == 0 and sb % c == 0
        self.nsb = b_core // sb
        self.nch = sb // c
        self.ncols = sb * NNODE       # node cols per super-block
        self.ecols = c * NEDGE        # edge cols per chunk

    def key(self):
        return (self.b_core, self.sb, self.c, self.mm_f32r, self.v2,
                tuple(sorted(self.zeros)), self.fnb, self.c_e,
                getattr(self, "rep", 1))


# ============================== v2 fast path ==============================

def build_program_v2(cfg: Cfg):
    nc = bacc.Bacc("TRN2", target_bir_lowering=False, debug=False)

    b, sb, c = cfg.b_core, cfg.sb, cfg.c
    ncols, ecols = cfg.ncols, cfg.ecols

    states_d = nc.dram_tensor("states", [b * NNODE, D], F32R,
                              kind="ExternalInput").ap()
    # weights arrive pre-transposed to [128, KT*H] so the DMA is a
    # contiguous per-partition row copy (128 descriptors, not 512)
    wnames = ["w_e1a", "w_e1b", "w_e2", "w_agg", "w_n1a", "w_n2", "w_n3"]
    wd = {n: nc.dram_tensor(n, [128, KT * H], BF16, kind="ExternalInput").ap()
          for n in wnames}
    vb_d = nc.dram_tensor("vecs_b", [128, 2 * KT], BF16,
                          kind="ExternalInput").ap()
    out_d = nc.dram_tensor("out", [b, NNODE + NEDGE], F32,
                           kind="ExternalOutput").ap()

    with tile.TileContext(nc) as tc, contextlib.ExitStack() as ctx, \
            nc.allow_low_precision(reason="bf16/fp8 pipeline, validated 7e-3"):
        singles = ctx.enter_context(tc.tile_pool(name="singles", bufs=1))
        p_node = ctx.enter_context(tc.tile_pool(name="p_node", bufs=2))
        p_ab1 = ctx.enter_context(tc.tile_pool(name="p_ab1", bufs=2))
        p_ab2 = ctx.enter_context(tc.tile_pool(name="p_ab2", bufs=1))
        p_sagg = ctx.enter_context(tc.tile_pool(name="p_sagg", bufs=1))
        p_node2 = ctx.enter_context(tc.tile_pool(name="p_node2", bufs=1))
        p_stage = ctx.enter_context(tc.tile_pool(name="p_stage", bufs=2))
        p_epre = ctx.enter_context(tc.tile_pool(name="p_epre", bufs=2))
        p_epre0 = ctx.enter_context(tc.tile_pool(name="p_epre0", bufs=2))
        p_h = ctx.enter_context(tc.tile_pool(name="p_h", bufs=2))
        p_sq = ctx.enter_context(tc.tile_pool(name="p_sq", bufs=2))
        p_tm = ctx.enter_context(tc.tile_pool(name="p_tm", bufs=2))
        p_bc = ctx.enter_context(tc.tile_pool(name="p_bc", bufs=2))
        p_red = ctx.enter_context(tc.tile_pool(name="p_red", bufs=2))
        p_small = ctx.enter_context(tc.tile_pool(name="p_small", bufs=2))
        p_out = ctx.enter_context(tc.tile_pool(name="p_out", bufs=2))
        ps_mm = ctx.enter_context(tc.tile_pool(name="ps_mm", bufs=3,
                                               space="PSUM"))
        ps_var = ctx.enter_context(tc.tile_pool(name="ps_var", bufs=2,
                                                space="PSUM"))

        # ---- constants ----
        ident_f = singles.tile([128, 128], F32)
        masks.make_identity(nc, ident_f[:, :])
        ident = singles.tile([128, 128], F32R)
        nc.vector.tensor_copy(ident, ident_f)

        ws = {}
        for n in wnames:
            ws[n] = singles.tile([128, KT, H], BF16, name=f"sb_{n}")
        vb = singles.tile([128, 2, KT], BF16)

        def load_weight(n):
            nc.sync.dma_start(
                out=ws[n], in_=wd[n].rearrange("p (k m) -> p k m", k=KT))

        # weight DMAs are emitted interleaved with the first super-block's
        # stage loads (below) so transposes start immediately; each DMA is
        # emitted before its first reader
        deferred_w = [["w_e1a", "w_e1b"],
                      ["w_e2", "w_agg", "w_n1a", "w_n2", "w_n3", "vb"]]
        # DoubleRow ldweights needs a >=32-wide m; rows 1-31 are unused
        one8_f = singles.tile([128, 2, 32], F32)
        nc.vector.memset(one8_f, 1.0)
        one8 = singles.tile([128, 2, 32], FP8)
        nc.vector.tensor_copy(one8, one8_f)
        eps_f = singles.tile([1, 2], F32)
        nc.vector.memset(eps_f[0:1, 0:1], LN_EPS * H * S_E1 * S_E1)
        nc.vector.memset(eps_f[0:1, 1:2], LN_EPS * H * S_E2 * S_E2)
        eps_r = singles.tile([1, 2], F32R)
        nc.vector.tensor_copy(eps_r, eps_f)
        ones_row_f = singles.tile([1, 448], F32)
        nc.vector.memset(ones_row_f, 1.0)
        ones_row = singles.tile([1, 448], F32R)
        nc.vector.tensor_copy(ones_row, ones_row_f)

        def load_w(name, k, m):
            return ws[name][:, k, m * 128:(m + 1) * 128]

        def mm_group(rhs_tiles_fn, wname, n, extra=None, mid_cb=None):
            """Per-m-pair accumulation: 2 two-bank PSUM tiles [128, 2, 512].
            mid_cb (if set) emits between the two pair sub-groups so short
            PE ops (var DR / head) interleave into the matmul stream."""
            psums = []
            for p in range(2):
                if p == 1 and mid_cb is not None:
                    mid_cb()
                ps = ps_mm.tile([128, 2, 512], F32, tag="mm", name=f"ps_mm{p}")
                for mi in range(2):
                    m = 2 * p + mi
                    for k in range(KT):
                        nc.tensor.matmul(
                            ps[:, mi, 0:n], load_w(wname, k, m),
                            rhs_tiles_fn(k),
                            start=(k == 0),
                            stop=(k == KT - 1 and extra is None))
                    if extra is not None:
                        wname2, rhs2_fn = extra
                        for k in range(KT):
                            nc.tensor.matmul(
                                ps[:, mi, 0:n], load_w(wname2, k, m),
                                rhs2_fn(k), start=False, stop=(k == KT - 1))
                psums.append(ps)
            return psums

        def edge_front(a_t, b_t, ch, epre0):
            c0 = ch * c * NNODE
            for k in range(KT):
                a_ap = (a_t[:, k, c0:c0 + c * NNODE]
                        .rearrange("p (g i) -> p g i", i=NNODE)
                        .unsqueeze(3).broadcast_to([128, c, NNODE, NNODE]))
                b_ap = (b_t[:, k, c0:c0 + c * NNODE]
                        .rearrange("p (g j) -> p g j", j=NNODE)
                        .unsqueeze(2).broadcast_to([128, c, NNODE, NNODE]))
                o_ap = epre0[:, k, :].rearrange("p (g i j) -> p g i j",
                                                i=NNODE, j=NNODE)
                nc.gpsimd.tensor_add(o_ap, a_ap, b_ap)

        def make_front(a_t, b_t, ch):
            epre0 = p_epre0.tile([128, KT, ecols], BF16, tag="epre0")
            edge_front(a_t, b_t, ch, epre0)
            epre = p_epre.tile([128, KT, ecols], BF16, tag="epre")
            nc.vector.tensor_scalar_max(epre, epre0, 0.0)
            return epre

        def compute_ab(src_t, pool, c0=0, n=None, tiles=None):
            n = ncols if n is None else n
            if tiles is None:
                a_t = pool.tile([128, KT, ncols], BF16, tag="a_t")
                b_t = pool.tile([128, KT, ncols], BF16, tag="b_t")
            else:
                a_t, b_t = tiles
            for wn, dst in [("w_e1a", a_t), ("w_e1b", b_t)]:
                psums = mm_group(
                    lambda k: src_t[:, k, c0:c0 + n], wn, n)
                nc.scalar.copy(dst[:, 0:2, c0:c0 + n], psums[0][:, :, 0:n])
                nc.scalar.copy(dst[:, 2:4, c0:c0 + n], psums[1][:, :, 0:n])
            return a_t, b_t

        def edge_evac(psums, sagg):
            """PSUM-evacuating ops (relu/sq) for one chunk."""
            n = ecols
            h_sb = p_h.tile([128, KT, ecols], BF16, tag="h")
            s_scale = S_E1 if sagg is not None else S_E2
            nc.scalar.activation(h_sb[:, 0:2, 0:n], psums[0][:, :, 0:n],
                                 AF.Relu)
            nc.vector.tensor_scalar_max(h_sb[:, 2:4, 0:n],
                                        psums[1][:, :, 0:n], 0.0)
            sq = p_sq.tile([128, KT, 448], FP8, tag="sq")
            nc.scalar.activation(sq[:, 0:2, 0:n], psums[0][:, :, 0:n],
                                 AF.Square, scale=s_scale)
            nc.scalar.activation(sq[:, 2:4, 0:n], psums[1][:, :, 0:n],
                                 AF.Square, scale=s_scale)
            return h_sb, sq

        def edge_late_pe(h_sb, sq, ch, sagg, out_head):
            """PE ops of a chunk tail (var DR + p2 head), injected mid-way
            into the next chunk's matmul group."""
            n = ecols
            psum_var = ps_var.tile([32, 512], F32, tag="var")
            for i in range(2):
                nc.tensor.matmul(psum_var[0:32, 0:n], one8,
                                 sq[:, 2 * i:2 * i + 2, 0:n],
                                 start=(i == 0), stop=(i == 1), perf_mode=DR)
            psum_eo = None
            if sagg is None:
                psum_eo = ps_var.tile([1, 512], F32, tag="var", name="ps_eo")
                for k in range(KT):
                    nc.tensor.matmul(psum_eo[0:1, 0:n], vb[:, 0, k:k + 1],
                                     h_sb[:, k, 0:n],
                                     start=(k == 0), stop=(k == KT - 1))
            return psum_var, psum_eo

        def edge_late_rest(psum_var, psum_eo, h_sb, ch, sagg, out_head):
            """Non-PE remainder of a chunk tail."""
            n = ecols
            inv = p_small.tile([1, 448], F32, tag="inv")
            nc.vector.reciprocal_approx_fast(inv[0:1, 0:n],
                                             psum_var[0:1, 0:n])
            rstd = p_small.tile([1, 448], BF16, tag="rstd")
            nc.scalar.activation(rstd[0:1, 0:n], inv[0:1, 0:n], AF.Sqrt)
            if sagg is not None:  # pass 1: tm = h*rstd, j-sum into sagg
                bc = p_bc.tile([128, 448], BF16, tag="bc")
                nc.gpsimd.partition_broadcast(bc[:, 0:n], rstd[0:1, 0:n],
                                              channels=128)
                tm = p_tm.tile([128, KT, ecols], BF16, tag="tm")
                nc.vector.tensor_tensor(
                    out=tm[:, :, 0:n], in0=h_sb[:, :, 0:n],
                    in1=bc[:, 0:n].unsqueeze(1).broadcast_to([128, KT, n]),
                    op=ALU.mult)
                # j-sum as an add tree: [x,7] -> (0:3)+(4:7) -> combine
                nx = c * NNODE
                tmj = tm.rearrange("p k (x j) -> p k x j", j=NNODE)
                s3 = p_red.tile([128, KT, nx, 3], BF16, tag="red3")
                nc.vector.tensor_add(s3, tmj[:, :, :, 0:3], tmj[:, :, :, 4:7])
                t2 = p_red.tile([128, KT, nx, 1], BF16, tag="red1a")
                nc.gpsimd.tensor_add(t2, s3[:, :, :, 0:1], s3[:, :, :, 1:2])
                t3 = p_red.tile([128, KT, nx, 1], BF16, tag="red1b")
                nc.gpsimd.tensor_add(t3, s3[:, :, :, 2:3], tmj[:, :, :, 3:4])
                sagg_v = sagg[:, :, ch * nx:(ch + 1) * nx].unsqueeze(3)
                nc.gpsimd.tensor_add(sagg_v, t2, t3)
            else:  # pass 2: eo = (v . h_sb) * rstd + c_e
                s, g0 = out_head
                eo_sb = p_out.tile([1, 448], F32, tag="head_sb")
                if cfg.c_e == 0.0:
                    nc.vector.tensor_tensor(out=eo_sb[0:1, 0:n],
                                            in0=psum_eo[0:1, 0:n],
                                            in1=rstd[0:1, 0:n], op=ALU.mult)
                else:
                    eo1 = p_out.tile([1, 448], F32, tag="eo1")
                    nc.vector.tensor_tensor(out=eo1[0:1, 0:n],
                                            in0=psum_eo[0:1, 0:n],
                                            in1=rstd[0:1, 0:n], op=ALU.mult)
                    nc.scalar.activation(eo_sb[0:1, 0:n], eo1[0:1, 0:n],
                                         AF.Copy, bias=cfg.c_e)
                nc.sync.dma_start(
                    out=out_d[g0:g0 + c, NNODE:NNODE + NEDGE].unsqueeze(0),
                    in_=eo_sb[0:1, 0:n].rearrange("o (g e) -> o g e",
                                                  e=NEDGE))

        # =========================== main loop ===========================
        for _rep in range(getattr(cfg, "rep", 1)):
          for s in range(cfg.nsb):
              # load + transpose states (f32r transpose, bf16 node_t)
              node_t = p_node.tile([128, KT, ncols], BF16, tag="node_t")
              a1_t = p_ab1.tile([128, KT, ncols], BF16, tag="a_t",
                                name="a1_t")
              b1_t = p_ab1.tile([128, KT, ncols], BF16, tag="b_t",
                                name="b1_t")
              r0 = s * ncols
              tcols = 112
              for t in range(ncols // tcols):
                  stg = p_stage.tile([tcols, D], F32R, tag="stage")
                  nc.sync.dma_start(
                      out=stg,
                      in_=states_d[r0 + t * tcols: r0 + (t + 1) * tcols, :])
                  psum_t = ps_mm.tile([128, 2, 512], F32R, tag="mm",
                                      name="ps_tp")
                  tpv = psum_t.rearrange("p a b -> p (a b)")[:, 0:KT * tcols] \
                      .rearrange("p (m q) -> p m q", q=tcols)
                  for m in range(KT):
                      nc.tensor.transpose(
                          tpv[:, m, :], stg[:, m * 128:(m + 1) * 128],
                          ident[0:tcols, 0:tcols])
                  nc.scalar.copy(
                      node_t[:, :, t * tcols:(t + 1) * tcols], tpv)
                  if deferred_w and t == 0:
                      for wn in deferred_w.pop(0):
                          if wn == "vb":
                              nc.sync.dma_start(
                                  out=vb, in_=vb_d.rearrange(
                                      "p (v k) -> p v k", v=2))
                          else:
                              load_weight(wn)
                  if t == (ncols // tcols) // 2 - 1:
                      compute_ab(node_t, p_ab1, c0=0, n=ncols // 2,
                                 tiles=(a1_t, b1_t))
              if deferred_w:
                  for wn in deferred_w.pop(0):
                      if wn == "vb":
                          nc.sync.dma_start(
                              out=vb,
                              in_=vb_d.rearrange("p (v k) -> p v k", v=2))
                      else:
                          load_weight(wn)
              compute_ab(node_t, p_ab1, c0=ncols // 2, n=ncols // 2,
                         tiles=(a1_t, b1_t))

              sagg = p_sagg.tile([128, KT, ncols], BF16, tag="sagg")
              epre = make_front(a1_t, b1_t, 0)
              pend = None
              for ch in range(cfg.nch):
                  late = [None]

                  def mid(pend=pend):
                      if pend is not None:
                          late[0] = edge_late_pe(*pend, sagg, None)

                  psums = mm_group(lambda k: epre[:, k, 0:ecols], "w_e2",
                                   ecols, mid_cb=mid)
                  if pend is not None:
                      edge_late_rest(*late[0], pend[0], pend[2], sagg, None)
                  h_sb, sq = edge_evac(psums, sagg)
                  if ch + 1 < cfg.nch:
                      epre = make_front(a1_t, b1_t, ch + 1)
                  pend = (h_sb, sq, ch)
              lv = edge_late_pe(*pend, sagg, None)
              edge_late_rest(*lv, pend[0], pend[2], sagg, None)

              # node MLP in two pipelined column halves (each half:
              # l1 -> relu -> l2 -> LN -> l3 -> head + its share of ab2)
              nhf = 224
              nq = ncols // nhf
              nh1 = p_epre.tile([128, KT, ncols], BF16, tag="epre",
                                name="nh1")
              for hf in range(nq):
                  c0 = hf * nhf
                  psums = mm_group(
                      lambda k, c0=c0: node_t[:, k, c0:c0 + nhf], "w_n1a",
                      nhf, extra=("w_agg",
                                  lambda k, c0=c0: sagg[:, k, c0:c0 + nhf]))
                  nc.scalar.activation(nh1[:, 0:2, c0:c0 + nhf],
                                       psums[0][:, :, 0:nhf], AF.Relu)
                  nc.vector.tensor_scalar_max(nh1[:, 2:4, c0:c0 + nhf],
                                              psums[1][:, :, 0:nhf], 0.0)

              nh_sb = p_h.tile([128, KT, ncols], BF16, tag="h", name="nh_sb")
              ntm = p_tm.tile([128, KT, ncols], BF16, tag="tm", name="ntm")
              nbc = p_bc.tile([128, ncols], BF16, tag="bc", name="nbc")
              node2_t = p_node2.tile([128, KT, ncols], BF16, tag="node2")
              a2_t = p_ab2.tile([128, KT, ncols], BF16, tag="a_t",
                                name="a2_t")
              b2_t = p_ab2.tile([128, KT, ncols], BF16, tag="b_t",
                                name="b2_t")

              def node_late(nsq, c0):
                  npsv = ps_var.tile([32, 512], F32, tag="var",
                                     name="npsv")
                  for i in range(2):
                      nc.tensor.matmul(npsv[0:32, 0:nhf], one8,
                                       nsq[:, 2 * i:2 * i + 2, 0:nhf],
                                       start=(i == 0), stop=(i == 1),
                                       perf_mode=DR)
                  ninv = p_small.tile([1, 448], F32, tag="inv", name="ninv")
                  nc.vector.reciprocal_approx_fast(ninv[0:1, 0:nhf],
                                                   npsv[0:1, 0:nhf])
                  nrstd = p_small.tile([1, 448], BF16, tag="rstd",
                                       name="nrstd")
                  nc.scalar.activation(nrstd[0:1, 0:nhf], ninv[0:1, 0:nhf],
                                       AF.Sqrt)
                  nc.gpsimd.partition_broadcast(nbc[:, c0:c0 + nhf],
                                                nrstd[0:1, 0:nhf],
                                                channels=128)
                  nc.vector.tensor_tensor(
                      out=ntm[:, :, c0:c0 + nhf],
                      in0=nh_sb[:, :, c0:c0 + nhf],
                      in1=nbc[:, c0:c0 + nhf].unsqueeze(1).broadcast_to(
                          [128, KT, nhf]),
                      op=ALU.mult)
                  psums = mm_group(
                      lambda k, c0=c0: ntm[:, k, c0:c0 + nhf], "w_n3", nhf)
                  nc.scalar.copy(node2_t[:, 0:2, c0:c0 + nhf],
                                 psums[0][:, :, 0:nhf])
                  nc.scalar.copy(node2_t[:, 2:4, c0:c0 + nhf],
                                 psums[1][:, :, 0:nhf])
                  # node head -> out[:, 0:7] for this half's graphs
                  psum_no = ps_var.tile([1, 512], F32, tag="var")
                  for k in range(KT):
                      nc.tensor.matmul(psum_no[0:1, 0:nhf],
                                       vb[:, 1, k:k + 1],
                                       node2_t[:, k, c0:c0 + nhf],
                                       start=(k == 0), stop=(k == KT - 1))
                  no_sb = p_out.tile([1, 448], F32, tag="head_sb")
                  nc.scalar.activation(no_sb[0:1, 0:nhf],
                                       psum_no[0:1, 0:nhf],
                                       AF.Copy, bias=cfg.fnb)
                  r0 = s * sb + (c0 // NNODE)
                  nc.sync.dma_start(
                      out=out_d[r0:r0 + nhf // NNODE, 0:NNODE].unsqueeze(0),
                      in_=no_sb[0:1, 0:nhf].rearrange("o (g i) -> o g i",
                                                      i=NNODE))
                  # this half's share of pass-2 A/B
                  compute_ab(node2_t, p_ab2, c0=c0, n=nhf,
                             tiles=(a2_t, b2_t))

              hf_graphs = nhf // NNODE
              pend_n = None
              for hf in range(nq):
                  c0 = hf * nhf
                  psums = mm_group(
                      lambda k, c0=c0: nh1[:, k, c0:c0 + nhf], "w_n2", nhf)
                  nc.scalar.activation(nh_sb[:, 0:2, c0:c0 + nhf],
                                       psums[0][:, :, 0:nhf], AF.Relu)
                  nc.vector.tensor_scalar_max(nh_sb[:, 2:4, c0:c0 + nhf],
                                              psums[1][:, :, 0:nhf], 0.0)
                  nsq = p_sq.tile([128, KT, 448], FP8, tag="sq",
                                  name=f"nsq{hf}")
                  nc.scalar.activation(nsq[:, 0:2, 0:nhf],
                                       psums[0][:, :, 0:nhf], AF.Square,
                                       scale=S_N)
                  nc.scalar.activation(nsq[:, 2:4, 0:nhf],
                                       psums[1][:, :, 0:nhf], AF.Square,
                                       scale=S_N)
                  if pend_n is not None:
                      node_late(*pend_n)
                  pend_n = (nsq, c0)
              node_late(*pend_n)

              # pass 2
              epre = make_front(a2_t, b2_t, 0)
              pend = None
              for ch in range(cfg.nch):
                  late = [None]

                  def mid(pend=pend):
                      if pend is not None:
                          late[0] = edge_late_pe(*pend, None,
                                                 (s, s * sb + pend[2] * c))

                  psums = mm_group(lambda k: epre[:, k, 0:ecols], "w_e2",
                                   ecols, mid_cb=mid)
                  if pend is not None:
                      edge_late_rest(*late[0], pend[0], pend[2], None,
                                     (s, s * sb + pend[2] * c))
                  h_sb, sq = edge_evac(psums, None)
                  if ch + 1 < cfg.nch:
                      epre = make_front(a2_t, b2_t, ch + 1)
                  pend = (h_sb, sq, ch)
              lv = edge_late_pe(*pend, None, (s, s * sb + pend[2] * c))
              edge_late_rest(*lv, pend[0], pend[2], None,
                             (s, s * sb + pend[2] * c))

    nc.compile()
    return nc


def host_fold_v2(inputs):
    f = lambda k: np.asarray(inputs[k], np.float64)
    ew1, ew2, ew3, eb3 = f("ew1"), f("ew2"), f("ew3"), f("eb3")
    nw1, nw2, nw3 = f("nw1"), f("nw2"), f("nw3")
    fnw, fnb, few, feb = f("fnw"), f("fnb"), f("few"), f("feb")

    ew2c = ew2 - ew2.mean(axis=1, keepdims=True)
    nw2c = nw2 - nw2.mean(axis=1, keepdims=True)
    C1 = S_E1 * np.sqrt(H)
    Cn = S_N * np.sqrt(H)
    C2 = S_E2 * np.sqrt(H)
    w_agg = (ew3 @ nw1[D:]) * C1
    v = (ew3 @ few)[:, 0] * C2
    c_e = float(eb3 @ few[:, 0] + feb[0])

    def g(x):
        """[D, H] -> [128, KT*H] with w_sb[p, k*H+m] = w[k*128+p, m]."""
        w = np.asarray(x, np.float32).reshape(KT, 128, H).transpose(1, 0, 2)
        return np.ascontiguousarray(w.reshape(128, KT * H)).astype(NPBF16)

    def gv(rows):
        """[2, H] -> [128, 2*KT] with vb[p, v*KT+k] = rows[v, k*128+p]."""
        r = np.asarray(rows, np.float32).reshape(2, KT, 128).transpose(2, 0, 1)
        return np.ascontiguousarray(r.reshape(128, 2 * KT)).astype(NPBF16)

    tensors = {
        "w_e1a": g(ew1[:D]), "w_e1b": g(ew1[D:]), "w_e2": g(ew2c),
        "w_agg": g(w_agg), "w_n1a": g(nw1[:D]), "w_n2": g(nw2c),
        "w_n3": g(nw3 * Cn),
        "vecs_b": gv(np.stack([v, fnw[:, 0]])),
    }
    return tensors, float(fnb[0]), c_e


def v2_ok(inputs):
    z = lambda k: not np.any(np.asarray(inputs[k]))
    o = lambda k: np.all(np.asarray(inputs[k]) == 1.0)
    return (z("eb1") and z("eb2") and z("ebt") and z("nb1") and z("nb2")
            and z("nbt") and z("nb3") and o("eg") and o("ng") and z("eb3"))


# ============================== v1 fallback ==============================

def build_program_v1(cfg: Cfg):
    """Generic path (arbitrary biases/gains), f32r matmuls."""
    nc = bacc.Bacc("TRN2", target_bir_lowering=False, debug=False)

    b, sb, c = cfg.b_core, cfg.sb, cfg.c
    ncols, ecols = cfg.ncols, cfg.ecols
    F32RM = mybir.dt.float32r if cfg.mm_f32r else F32

    states_d = nc.dram_tensor("states", [b * NNODE, D], F32, kind="ExternalInput").ap()
    wnames = ["w_e1a", "w_e1b", "w_e2", "w_agg", "w_n1a", "w_n2", "w_n3"]
    wd = {n: nc.dram_tensor(n, [D, H], F32RM, kind="ExternalInput").ap()
          for n in wnames}
    vecs_d = nc.dram_tensor("vecs_kt", [NV, H], F32, kind="ExternalInput").ap()
    vecsr_d = nc.dram_tensor("vecs_r", [NV, H], F32RM, kind="ExternalInput").ap()
    out_d = nc.dram_tensor("out", [b, NNODE + NEDGE], F32, kind="ExternalOutput").ap()

    with tile.TileContext(nc) as tc, contextlib.ExitStack() as ctx:
        singles = ctx.enter_context(tc.tile_pool(name="singles", bufs=1))
        p_node = ctx.enter_context(tc.tile_pool(name="p_node", bufs=2))
        p_ab1 = ctx.enter_context(tc.tile_pool(name="p_ab1", bufs=2))
        p_ab2 = ctx.enter_context(tc.tile_pool(name="p_ab2", bufs=1))
        p_sagg = ctx.enter_context(tc.tile_pool(name="p_sagg", bufs=1))
        p_node2 = ctx.enter_context(tc.tile_pool(name="p_node2", bufs=1))
        p_stage = ctx.enter_context(tc.tile_pool(name="p_stage", bufs=2))
        p_epre = ctx.enter_context(tc.tile_pool(name="p_epre", bufs=2))
        fast_build = all(i in cfg.zeros_idx for i in (V_EB2, V_EBT, V_NB2, V_NBT))
        p_epre0 = ctx.enter_context(
            tc.tile_pool(name="p_epre0", bufs=2 if fast_build else 1))
        p_h = ctx.enter_context(tc.tile_pool(name="p_h", bufs=2))
        p_sq = ctx.enter_context(tc.tile_pool(name="p_sq", bufs=1))
        p_tm = ctx.enter_context(tc.tile_pool(name="p_tm", bufs=1))
        p_bc = ctx.enter_context(tc.tile_pool(name="p_bc", bufs=2))
        p_small = ctx.enter_context(tc.tile_pool(name="p_small", bufs=2))
        p_out = ctx.enter_context(tc.tile_pool(name="p_out", bufs=1))
        ps_mm = ctx.enter_context(tc.tile_pool(name="ps_mm", bufs=2, space="PSUM"))
        ps_var = ctx.enter_context(tc.tile_pool(name="ps_var", bufs=2, space="PSUM"))
        ps_bc = ctx.enter_context(tc.tile_pool(name="ps_bc", bufs=2, space="PSUM"))

        ident = singles.tile([128, 128], F32)
        masks.make_identity(nc, ident[:, :])

        ws = {}
        for n in wnames:
            wt = singles.tile([128, KT, H], F32RM, name=f"sb_{n}")
            nc.sync.dma_start(out=wt, in_=wd[n].rearrange("(k p) m -> p k m", p=128))
            ws[n] = wt

        vecs = singles.tile([128, NV, KT], F32)
        nc.sync.dma_start(out=vecs, in_=vecs_d.rearrange("v (k p) -> p v k", p=128))
        vecs_r = singles.tile([128, NV, KT], F32RM)
        nc.sync.dma_start(out=vecs_r,
                          in_=vecsr_d.rearrange("v (k p) -> p v k", p=128))
        ones_f = singles.tile([1, 128], F32)
        nc.vector.memset(ones_f, 1.0)
        ones_col = singles.tile([1, 128], F32RM)
        nc.vector.tensor_copy(ones_col, ones_f)
        eps_t = singles.tile([1, 1], F32)
        nc.vector.memset(eps_t, LN_EPS)
        eps_r = singles.tile([1, 1], F32RM)
        nc.vector.tensor_copy(eps_r, eps_t)
        ones_row_f = singles.tile([1, 448], F32)
        nc.vector.memset(ones_row_f, 1.0)
        ones_row = singles.tile([1, 448], F32RM)
        nc.vector.tensor_copy(ones_row, ones_row_f)

        def load_w(name, k, m):
            return ws[name][:, k, m * 128:(m + 1) * 128]

        def vslice(v, k):
            return vecs_r[:, v, k:k + 1]

        def mm_group(rhs_tiles_fn, wname, n, extra=None):
            psums = []
            for p in range(2):
                ps = ps_mm.tile([128, 2, 512], F32, tag="mm", name=f"ps_mm{p}")
                for mi in range(2):
                    m = 2 * p + mi
                    for k in range(KT):
                        nc.tensor.matmul(
                            ps[:, mi, 0:n], load_w(wname, k, m), rhs_tiles_fn(k),
                            start=(k == 0), stop=(k == KT - 1 and extra is None))
                    if extra is not None:
                        wname2, rhs2_fn = extra
                        for k in range(KT):
                            nc.tensor.matmul(
                                ps[:, mi, 0:n], load_w(wname2, k, m), rhs2_fn(k),
                                start=False, stop=(k == KT - 1))
                psums.append(ps)
            return psums

        def ln_tail(psums, n, wss_idx, bias_idx, relu_bias_idx, h_sb, sq, bc_sb):
            fast = (bias_idx in cfg.zeros_idx
                    and relu_bias_idx in cfg.zeros_idx)
            for p in range(2):
                hv = h_sb[:, 2 * p:2 * p + 2, 0:n]
                pv = psums[p][:, :, 0:n]
                if fast:
                    if p == 0:
                        nc.scalar.activation(hv, pv, AF.Relu)
                    else:
                        nc.vector.tensor_scalar_max(hv, pv, 0.0)
                    nc.scalar.square(sq[:, 2 * p:2 * p + 2, 0:n], pv)
                else:
                    for mi in range(2):
                        m = 2 * p + mi
                        nc.vector.tensor_scalar_add(
                            h_sb[:, m, 0:n], psums[p][:, mi, 0:n],
                            vecs[:, bias_idx, m:m + 1])
                    nc.scalar.square(sq[:, 2 * p:2 * p + 2, 0:n],
                                     h_sb[:, 2 * p:2 * p + 2, 0:n])
            psum_var = ps_var.tile([1, 512], F32, tag="var")
            for k in range(KT):
                nc.tensor.matmul(
                    psum_var[0:1, 0:n], vslice(wss_idx, k), sq[:, k, 0:n],
                    start=(k == 0), stop=False)
            nc.tensor.matmul(psum_var[0:1, 0:n], eps_r[0:1, 0:1],
                             ones_row[0:1, 0:n], start=False, stop=True)
            inv_v = p_small.tile([1, 448], F32, tag="s_sb", bufs=1)
            nc.vector.reciprocal_approx_fast(inv_v[0:1, 0:n],
                                             psum_var[0:1, 0:n])
            rstd_r = p_small.tile([1, 448], F32RM, tag="rstd_r")
            nc.scalar.activation(rstd_r[0:1, 0:n], inv_v[0:1, 0:n], AF.Sqrt)
            psum_b = ps_bc.tile([128, 512], F32, tag="bc")
            nc.tensor.matmul(psum_b[:, 0:n], ones_col[0:1, :],
                             rstd_r[0:1, 0:n], start=True, stop=True)
            nc.scalar.copy(bc_sb[:, 0:n], psum_b[:, 0:n])
            return fast

        def ln_apply(fast, h_sb, bc_sb, n, relu_bias_idx, out_tile):
            bcb = bc_sb.unsqueeze(1).broadcast_to([128, KT, n])
            if fast:
                nc.gpsimd.tensor_mul(out_tile[:, :, 0:n], h_sb[:, :, 0:n], bcb)
            else:
                tmf = p_tm.tile([128, KT, 448], F32, tag="tmf")
                nc.vector.tensor_mul(tmf[:, :, 0:n], h_sb[:, :, 0:n], bcb)
                for k in range(KT):
                    nc.vector.tensor_scalar(
                        out=out_tile[:, k, 0:n], in0=tmf[:, k, 0:n],
                        scalar1=vecs[:, relu_bias_idx, k:k + 1], scalar2=0.0,
                        op0=ALU.add, op1=ALU.max)

        def edge_front(a_t, b_t, ch, dst_pre):
            c0 = ch * c * NNODE
            epre0 = p_epre0.tile([128, KT, ecols], F32, tag="epre0")
            for k in range(KT):
                a_ap = (a_t[:, k, c0:c0 + c * NNODE]
                        .rearrange("p (g i) -> p g i", i=NNODE)
                        .unsqueeze(3).broadcast_to([128, c, NNODE, NNODE]))
                b_ap = (b_t[:, k, c0:c0 + c * NNODE]
                        .rearrange("p (g j) -> p g j", j=NNODE)
                        .unsqueeze(2).broadcast_to([128, c, NNODE, NNODE]))
                o_ap = epre0[:, k, :].rearrange("p (g i j) -> p g i j",
                                                i=NNODE, j=NNODE)
                eng = nc.vector if k < 2 else nc.gpsimd
                eng.tensor_add(o_ap, a_ap, b_ap)
            nc.gpsimd.tensor_scalar_max(dst_pre, epre0, 0.0)

        def compute_ab(src_t, pool):
            a_t = pool.tile([128, KT, ncols], F32RM, tag="a_t")
            b_t = pool.tile([128, KT, ncols], F32RM, tag="b_t")
            for wn, dst, bias_idx in [("w_e1a", a_t, V_EB1), ("w_e1b", b_t, None)]:
                psums = mm_group(lambda k: src_t[:, k, 0:ncols], wn, ncols)
                for p in range(2):
                    dv = dst[:, 2 * p:2 * p + 2, 0:ncols]
                    pv = psums[p][:, :, 0:ncols]
                    if bias_idx is not None and bias_idx not in cfg.zeros_idx:
                        for mi in range(2):
                            m = 2 * p + mi
                            nc.vector.tensor_scalar_add(
                                dst[:, m, 0:ncols], psums[p][:, mi, 0:ncols],
                                vecs[:, bias_idx, m:m + 1])
                    elif p == 0:
                        nc.scalar.copy(dv, pv)
                    else:
                        nc.vector.tensor_copy(dv, pv)
            return a_t, b_t

        def make_front(a_t, b_t, ch):
            epre = p_epre.tile([128, KT, ecols], F32RM, tag="epre")
            edge_front(a_t, b_t, ch, epre)
            return epre

        def edge_chunk(epre, front_next, ch, sagg, out_head):
            psums = mm_group(lambda k: epre[:, k, 0:ecols], "w_e2", ecols)
            nxt = front_next() if front_next else None
            h_sb = p_h.tile([128, KT, ecols], F32, tag="h")
            sq = p_sq.tile([128, KT, ecols], F32RM, tag="sq")
            bc_sb = p_bc.tile([128, ecols], F32, tag="bcs")
            fast = ln_tail(psums, ecols, V_WSSE, V_EB2, V_EBT, h_sb, sq, bc_sb)
            if sagg is not None:
                tm = p_tm.tile([128, KT, ecols], F32RM, tag="tm1")
                ln_apply(fast, h_sb, bc_sb, ecols, V_EBT, tm)
                with nc.allow_low_precision(reason="f32r round of f32 sum"):
                    nc.vector.tensor_reduce(
                        sagg[:, :, ch * c * NNODE:(ch + 1) * c * NNODE],
                        tm.rearrange("p k (n j) -> p k n j", j=NNODE),
                        axis=mybir.AxisListType.X, op=ALU.add)
            else:
                tm = p_tm.tile([128, KT, ecols], F32RM, tag="tm2")
                ln_apply(fast, h_sb, bc_sb, ecols, V_EBT, tm)
                s, g0 = out_head
                psum_eo = ps_var.tile([1, 512], F32, tag="var")
                for k in range(KT):
                    nc.tensor.matmul(psum_eo[0:1, 0:ecols], vslice(V_V, k),
                                     tm[:, k, 0:ecols],
                                     start=(k == 0), stop=(k == KT - 1))
                eo_sb = p_out.tile([1, 448], F32, tag="head_sb")
                nc.scalar.activation(eo_sb[0:1, 0:ecols], psum_eo[0:1, 0:ecols],
                                     AF.Copy, bias=cfg.c_e)
                nc.sync.dma_start(
                    out=out_d[g0:g0 + c, NNODE:NNODE + NEDGE].unsqueeze(0),
                    in_=eo_sb[0:1, 0:ecols].rearrange("o (g e) -> o g e",
                                                      e=NEDGE))
            return nxt

        for _rep in range(getattr(cfg, "rep", 1)):
          for s in range(cfg.nsb):
              node_t = p_node.tile([128, KT, ncols], F32RM, tag="node_t")
              r0 = s * ncols
              tcols = 112
              for t in range(ncols // tcols):
                  stg = p_stage.tile([tcols, D], F32, tag="stage")
                  nc.sync.dma_start(
                      out=stg, in_=states_d[r0 + t * tcols: r0 + (t + 1) * tcols, :])
                  psum_t = ps_mm.tile([128, 2, 512], F32, tag="mm", name="ps_tp")
                  tpv = psum_t.rearrange("p a b -> p (a b)")[:, 0:KT * tcols] \
                      .rearrange("p (m q) -> p m q", q=tcols)
                  for m in range(KT):
                      nc.tensor.transpose(
                          tpv[:, m, :], stg[:, m * 128:(m + 1) * 128],
                          ident[0:tcols, 0:tcols])
                  nc.scalar.copy(
                      node_t[:, :, t * tcols:(t + 1) * tcols], tpv)

              a1_t, b1_t = compute_ab(node_t, p_ab1)

              sagg = p_sagg.tile([128, KT, ncols], F32RM, tag="sagg")
              epre = make_front(a1_t, b1_t, 0)
              for ch in range(cfg.nch):
                  nf = ((lambda cc=ch: make_front(a1_t, b1_t, cc + 1))
                        if ch + 1 < cfg.nch else None)
                  epre = edge_chunk(epre, nf, ch, sagg, None)

              psums = mm_group(lambda k: node_t[:, k, 0:ncols], "w_n1a", ncols,
                               extra=("w_agg", lambda k: sagg[:, k, 0:ncols]))
              nh1 = p_epre.tile([128, KT, ncols], F32RM, tag="epre")
              for p in range(2):
                  nv = nh1[:, 2 * p:2 * p + 2, 0:ncols]
                  pv = psums[p][:, :, 0:ncols]
                  if V_NB1 in cfg.zeros_idx:
                      if p == 0:
                          nc.scalar.activation(nv, pv, AF.Relu)
                      else:
                          nc.vector.tensor_scalar_max(nv, pv, 0.0)
                  else:
                      for mi in range(2):
                          m = 2 * p + mi
                          nc.scalar.activation(
                              nh1[:, m, 0:ncols], psums[p][:, mi, 0:ncols],
                              AF.Relu, bias=vecs[:, V_NB1, m:m + 1])

              psums = mm_group(lambda k: nh1[:, k, 0:ncols], "w_n2", ncols)
              nh_sb = p_h.tile([128, KT, ncols], F32, tag="h")
              nsq = p_sq.tile([128, KT, ncols], F32RM, tag="sq")
              nbc = p_bc.tile([128, ncols], F32, tag="bcs")
              nfast = ln_tail(psums, ncols, V_WSSN, V_NB2, V_NBT, nh_sb, nsq, nbc)
              ntm = p_tm.tile([128, KT, ncols], F32RM, tag="tm2")
              ln_apply(nfast, nh_sb, nbc, ncols, V_NBT, ntm)

              psums = mm_group(lambda k: ntm[:, k, 0:ncols], "w_n3", ncols)
              node2_t = p_node2.tile([128, KT, ncols], F32RM, tag="node2")
              for p in range(2):
                  nv = node2_t[:, 2 * p:2 * p + 2, 0:ncols]
                  pv = psums[p][:, :, 0:ncols]
                  if V_NB3 in cfg.zeros_idx:
                      if p == 0:
                          nc.scalar.copy(nv, pv)
                      else:
                          nc.vector.tensor_copy(nv, pv)
                  else:
                      for mi in range(2):
                          m = 2 * p + mi
                          nc.vector.tensor_scalar_add(
                              node2_t[:, m, 0:ncols], psums[p][:, mi, 0:ncols],
                              vecs[:, V_NB3, m:m + 1])

              psum_no = ps_var.tile([1, 512], F32, tag="var")
              for k in range(KT):
                  nc.tensor.matmul(psum_no[0:1, 0:ncols], vslice(V_FNW, k),
                                   node2_t[:, k, 0:ncols],
                                   start=(k == 0), stop=(k == KT - 1))
              no_sb = p_out.tile([1, 448], F32, tag="head_sb")
              nc.scalar.activation(no_sb[0:1, 0:ncols], psum_no[0:1, 0:ncols],
                                   AF.Copy, bias=cfg.fnb)
              nc.sync.dma_start(
                  out=out_d[s * sb:(s + 1) * sb, 0:NNODE].unsqueeze(0),
                  in_=no_sb[0:1, 0:ncols].rearrange("o (g i) -> o g i", i=NNODE))

              a2_t, b2_t = compute_ab(node2_t, p_ab2)
              epre = make_front(a2_t, b2_t, 0)
              for ch in range(cfg.nch):
                  nf = ((lambda cc=ch: make_front(a2_t, b2_t, cc + 1))
                        if ch + 1 < cfg.nch else None)
                  epre = edge_chunk(epre, nf, ch, None, (s, s * sb + ch * c))

    nc.compile()
    return nc


def host_fold_v1(inputs):
    f = lambda k: np.asarray(inputs[k], np.float64)
    ew1, eb1, ew2, eb2 = f("ew1"), f("eb1"), f("ew2"), f("eb2")
    eg, ebt, ew3, eb3 = f("eg"), f("ebt"), f("ew3"), f("eb3")
    nw1, nb1, nw2, nb2 = f("nw1"), f("nb1"), f("nw2"), f("nb2")
    ng, nbt, nw3, nb3 = f("ng"), f("nbt"), f("nw3"), f("nb3")
    fnw, fnb, few, feb = f("fnw"), f("fnb"), f("few"), f("feb")

    ew2c = ew2 - ew2.mean(axis=1, keepdims=True)
    eb2cg = (eb2 - eb2.mean()) * eg
    ew2cg = ew2c * eg[None, :]
    wss_e = 1.0 / np.maximum(eg * eg, 1e-12) / H

    nw1a, nw1b = nw1[:D], nw1[D:]
    w_agg = ew3 @ nw1b
    nb1p = nb1 + 7.0 * (eb3 @ nw1b)
    nw2c = nw2 - nw2.mean(axis=1, keepdims=True)
    nb2cg = (nb2 - nb2.mean()) * ng
    nw2cg = nw2c * ng[None, :]
    wss_n = 1.0 / np.maximum(ng * ng, 1e-12) / H

    v = (ew3 @ few)[:, 0]
    c_e = float(eb3 @ few[:, 0] + feb[0])

    g = lambda x: np.ascontiguousarray(x, np.float32)
    vec_rows = [eb1, ebt, wss_e, nbt, wss_n, v, fnw[:, 0], eb2cg, nb1p, nb2cg, nb3]
    vecs_kt = g(np.stack(vec_rows))

    zeros = frozenset(
        i for i in (V_EB1, V_EBT, V_NBT, V_EB2, V_NB1, V_NB2, V_NB3)
        if not np.any(vec_rows[i]))

    tensors = {
        "w_e1a": g(ew1[:D]), "w_e1b": g(ew1[D:]), "w_e2": g(ew2cg),
        "w_agg": g(w_agg), "w_n1a": g(nw1a), "w_n2": g(nw2cg), "w_n3": g(nw3),
        "vecs_kt": vecs_kt, "vecs_r": vecs_kt,
    }
    return tensors, zeros, float(fnb[0]), c_e


_CACHE = {}


def get_program(cfg: Cfg):
    cfg.zeros_idx = cfg.zeros
    key = cfg.key()
    if key not in _CACHE:
        _CACHE[key] = (build_program_v2(cfg) if cfg.v2
                       else build_program_v1(cfg))
    return _CACHE[key]


def prepare(inputs, b_core):
    """Returns (cfg, folded-tensors dict)."""
    if v2_ok(inputs):
        folded, fnb, c_e = host_fold_v2(inputs)
        cfg = Cfg(b_core=b_core, sb=128, fnb=fnb, c_e=c_e, v2=True)
    else:
        folded, zeros, fnb, c_e = host_fold_v1(inputs)
        cfg = Cfg(b_core=b_core, zeros=zeros, fnb=fnb, c_e=c_e)
    return cfg, folded


def kernel(**inputs) -> np.ndarray:
    states = np.asarray(inputs["states"], np.float32)
    B, n, d = states.shape
    assert (B, n, d) == (B_FULL, NNODE, D)

    cfg, folded = prepare(inputs, B // N_CORES)
    nc = get_program(cfg)

    in_maps = []
    for ci in range(N_CORES):
        m = dict(folded)
        m["states"] = np.ascontiguousarray(
            states[ci * cfg.b_core:(ci + 1) * cfg.b_core].reshape(-1, D))
        in_maps.append(m)

    res = run_bass_kernel_spmd(nc, in_maps, list(range(N_CORES)))
    return np.concatenate([r["out"] for r in res.results], axis=0)
